# revision 51
# baseline (speedup 1.0000x reference)
"""AdaptDHM MoE-routing kernel for one TRN2 chip (8 NeuronCores).

Strategy (expert-parallel dispatch, done host-side):
  - router = argmax(x @ center.T) picks one of C=8 clusters per token.
  - Each token is dispatched to the core owning its cluster; the core runs
    the 4-layer MLP (1024->2048->1024->512->1, relu; final sigmoid on
    host) once per token, padded to common capacity K (SPMD, one NEFF).
  - Compute: layers 0-2 in fp8-e4m3 with DoubleRow matmuls, layer 3 in
    bf16 token-major (tokens as lhsT columns, out free dim 1) so its PE
    cost is negligible and the output lands partition-major (tiny DMA).
  - Schedule (raw builder, manual semaphores, no Tile barriers): K split
    into (512, 512, rem) token tiles, layers software-pipelined in a
    wavefront (A0 B0 | A1+C0 | B1+C1 | A2+C2 | B2 | L3s) so the tensor
    engine never waits for a relu at a layer boundary. Warm-up matmuls on
    zeroed scratch keep the PE p-state ramp hot while the first DMAs
    land; weight/x DMAs are ordered to match consumption so the PE is
    supply-fed throughout. Relus alternate Scalar/Vector engines with an
    explicit balanced assignment in the tail chunks.
  - Output: SWDGE descriptors for the [128, NTB] logits are prepared at
    kernel start (kv_writeback prepare_only, ctx_idx=0 == plain
    SBUF->DRAM write); the end-of-kernel trigger skips the HWDGE+DGE
    issue latency on the critical tail.
  - Host scatters per-core logits back to the [B] output and applies the
    sigmoid.
"""

import math
import os

import ml_dtypes
import numpy as np

B, DIMS = 8192, 1024
FCN = [DIMS, 2048, 1024, 512, 1]
C = 8
NCORES = 8
P = 128
TT = 512  # max token tile (matmul moving free dim / PSUM bank)

_BF16 = ml_dtypes.bfloat16

_graph_cache = {}
last_run = None  # BassKernelResults of the most recent kernel() call

# per-layer (in_blocks, out_blocks)
_LAYER_BLOCKS = [(8, 16), (16, 8), (8, 4), (4, 1)]
# out columns per DMA-able weight block
_WBLK_OCOLS = [256, 128, 512, 1]

NDUMMY = int(os.environ.get("KERNEL_NDUMMY", "42"))


def _token_tiles(K):
    """Split K into full TT-sized tiles plus one remainder tile."""
    assert K % 16 == 0
    tiles = []
    t0 = 0
    while K - t0 >= TT:
        tiles.append((t0, TT))
        t0 += TT
    if K - t0 > 0:
        tiles.append((t0, K - t0))
    return tiles


def _tile_tblocks(tiles):
    """Token-blocks (<=128 tokens) per tile for the token-major L3.

    Returns list over tiles of list of (col, c_off, ncols); col is the
    global output column index in the [P, NTB] output."""
    out = []
    col = 0
    for (t0, tsz) in tiles:
        blocks = []
        c_off = 0
        while c_off < tsz:
            n = min(P, tsz - c_off)
            blocks.append((col, c_off, n))
            col += 1
            c_off += n
        out.append(blocks)
    return out, col


def _build_graph_raw(K, c0, c1, c2, ndummy=None):
    """Raw (no-Tile) builder: manual semaphores, no entry/exit barriers,
    SWDGE prepare/trigger output path. V8 wavefront schedule; requires
    nt == 3 with a small remainder tile."""
    import concourse.bass as bass  # noqa: F401
    from contextlib import ExitStack
    from concourse import bacc, mybir

    if ndummy is None:
        ndummy = NDUMMY
    f8 = mybir.dt.float8e4
    bf = mybir.dt.bfloat16
    f32 = mybir.dt.float32
    i32 = mybir.dt.int32
    AF = mybir.ActivationFunctionType
    DR = mybir.MatmulPerfMode.DoubleRow
    wdt = [f8, f8, f8, bf]

    nc = bacc.Bacc("TRN2", target_bir_lowering=False, debug=False,
                   num_devices=NCORES)

    tiles = _token_tiles(K)
    nt = len(tiles)
    assert nt == 3 and tiles[2][1] <= 256
    tblocks, NTB = _tile_tblocks(tiles)
    rem = tiles[2][1]

    xT_d = [nc.declare_dram_parameter(f"xT{ti}", [P, 8, tsz], f8, False)
            for ti, (t0, tsz) in enumerate(tiles)]
    w_d = []
    for li, (ib, ob) in enumerate(_LAYER_BLOCKS):
        ocols = _WBLK_OCOLS[li]
        nblk = (ob * P) // ocols if li < 3 else 1
        w_d.append(nc.declare_dram_parameter(
            f"w{li}", [nblk, P, ib, ocols], wdt[li], False))
    out_d = nc.declare_dram_parameter("out", [1, P, 1, NTB], f32, True)

    sp, pe, act, dve, gp = nc.sync, nc.tensor, nc.scalar, nc.vector, nc.gpsimd
    NPS = 6  # rotating psum banks

    with ExitStack() as ctx:
        sb = lambda name, shape, dt: ctx.enter_context(
            nc.sbuf_tensor(name, shape, dt))
        w0s = sb("w0s", [P, 8, 8, 256], f8)
        w1s = sb("w1s", [P, 8, 16, 128], f8)
        w2s = sb("w2s", [P, 8, 512], f8)
        w3s = sb("w3s", [P, 4, 1], bf)
        xsb = [sb(f"xs{ti}", [P, 8, tsz], f8)
               for ti, (t0, tsz) in enumerate(tiles)]
        h1 = [sb(f"h1_{ti}", [P, 16, tsz], f8)
              for ti, (t0, tsz) in enumerate(tiles)]
        h2 = [sb(f"h2_{ti}", [P, 8, tsz], f8)
              for ti, (t0, tsz) in enumerate(tiles)]
        h3 = [sb(f"h3_{ti}", [P, 4, tsz], bf)
              for ti, (t0, tsz) in enumerate(tiles)]
        outs = sb("outs", [P, 1, 1, NTB], f32)
        zs = sb("zs", [P, 2, 320], f8)
        kvidx = sb("kvidx", [P, 1], i32)
        banks = [ctx.enter_context(nc.psum_tensor(f"pb{i}", [P, TT], f32))
                 for i in range(NPS)]
        zp = ctx.enter_context(nc.psum_tensor("zp", [P, 192], f32))
        ps3 = ctx.enter_context(nc.psum_tensor("ps3", [P, 16], f32))

        sem = lambda name: ctx.enter_context(nc.semaphore(name))
        zs_sem = sem("zs")
        pe_sem = sem("pe")
        act_sem = sem("act")
        dve_sem = sem("dve")
        copy_sem = sem("copy")
        prep_sem = sem("prep")
        odma_sem = sem("odma")

        # ---- SP: input DMAs in consumption order ----
        dma_order = [("w0", 0), ("x", 0)]
        dma_order += [("w0", b) for b in range(1, 8)]
        dma_order += [("x", 1), ("x", 2)]
        dma_order += [("w1", b) for b in range(8)]
        dma_order += [("w2", 0), ("w3", 0)]
        dsem = {}
        for key in dma_order:
            kind, b = key
            dsem[key] = sem(f"dma_{kind}_{b}")
            if kind == "x":
                dst, src = xsb[b][:], xT_d[b][:]
            elif kind == "w0":
                dst, src = w0s[:, b], w_d[0][b]
            elif kind == "w1":
                dst, src = w1s[:, b], w_d[1][b]
            elif kind == "w2":
                dst, src = w2s[:], w_d[2][0]
            else:
                dst, src = w3s[:], w_d[3][0]
            sp.dma_start(out=dst, in_=src).then_inc(dsem[key], 16)

        # ---- Pool: output-path prep; trigger at the end ----
        gp.memset(kvidx[:], 0)
        gp.kv_writeback(out_d[:], outs[:], kvidx[:],
                        prepare_only=True, sem=odma_sem
                        ).then_inc(prep_sem, 1)

        # ---- DVE: scratch/ps3 memsets ----
        dve.memset(zs[:], 0.0).then_inc(zs_sem, 1)
        dve.memset(ps3[:], 0.0).then_inc(zs_sem, 1)

        # ---- PE warm-up ----
        pe.wait_ge(zs_sem, 1)
        for _ in range(ndummy):
            pe.matmul(zp[:], zs[:, :, 0:128], zs[:, :, 128:320],
                      start=True, stop=True, perf_mode=DR)

        # ---- main pipeline (V8 order) ----
        waited = {"pe": {}, "act": {}, "dve": {}}

        def wait(eng_obj, eng_key, s, v):
            w = waited[eng_key]
            if w.get(s.name, -1) < v:
                eng_obj.wait_ge(s, v)
                w[s.name] = v

        relu_cnt = [0]
        cnts = {"a": 0, "d": 0}
        group_relu = {}
        gidx = [0]
        hts = [None, h1, h2, h3]
        scales = [c0, c1, c2]

        def wslice(li, o, k):
            if li == 0:
                return w0s[:, o // 2, 2 * k:2 * k + 2,
                           (o % 2) * P:(o % 2) * P + P]
            if li == 1:
                return w1s[:, o, 2 * k:2 * k + 2, :]
            return w2s[:, 2 * k:2 * k + 2, o * P:(o + 1) * P]

        def wait_relu(eng_obj, eng_key, key):
            e, c = group_relu[key]
            if e == "s":
                wait(eng_obj, eng_key, act_sem, c[0])
                wait(eng_obj, eng_key, dve_sem, c[1])
            else:
                wait(eng_obj, eng_key, act_sem if e == "a" else dve_sem, c)

        def group(ti, li, o, eng=None):
            gi = gidx[0]
            t0_, tsz = tiles[ti]
            npair = [4, 8, 4][li]
            ps = banks[gi % NPS]
            # PSUM WAR: slot reused from group gi-NPS
            if gi >= NPS:
                wait_relu(pe, "pe", gkeys[gi - NPS])
            # weight/x DMA deps
            if li == 0:
                wait(pe, "pe", dsem[("x", ti)], 16)
                wait(pe, "pe", dsem[("w0", o // 2)], 16)
            elif li == 1:
                wait(pe, "pe", dsem[("w1", o)], 16)
            else:
                wait(pe, "pe", dsem[("w2", 0)], 16)
            rhs_t = xsb[ti] if li == 0 else hts[li][ti]
            for k in range(npair):
                if li > 0:
                    for bo in (2 * k, 2 * k + 1):
                        wait_relu(pe, "pe", (ti, li - 1, bo))
                mm = pe.matmul(ps[:, :tsz], wslice(li, o, k),
                               rhs_t[:, 2 * k:2 * k + 2, :tsz],
                               start=(k == 0), stop=(k == npair - 1),
                               perf_mode=DR)
                if k == npair - 1:
                    mm.then_inc(pe_sem, 1)
            # relu; eng "s" = token-split across both engines (low latency)
            if eng is None:
                eng = "a" if relu_cnt[0] % 2 == 0 else "d"
            relu_cnt[0] += 1
            dst = hts[li + 1][ti][:, o, :tsz]
            if eng == "s":
                h = tsz // 2 // 16 * 16
                cnts["a"] += 1
                cnts["d"] += 1
                group_relu[(ti, li, o)] = ("s", (cnts["a"], cnts["d"]))
                wait(act, "act", pe_sem, gi + 1)
                act.activation(dst[:, :h], ps[:, :h], AF.Relu,
                               scale=scales[li]).then_inc(act_sem, 1)
                wait(dve, "dve", pe_sem, gi + 1)
                dve.tensor_scalar(dst[:, h:tsz], ps[:, h:tsz], scales[li],
                                  0.0, mybir.AluOpType.mult,
                                  mybir.AluOpType.max).then_inc(dve_sem, 1)
            elif eng == "a":
                cnts[eng] += 1
                group_relu[(ti, li, o)] = (eng, cnts[eng])
                wait(act, "act", pe_sem, gi + 1)
                act.activation(dst, ps[:, :tsz], AF.Relu,
                               scale=scales[li]).then_inc(act_sem, 1)
            else:
                cnts[eng] += 1
                group_relu[(ti, li, o)] = (eng, cnts[eng])
                wait(dve, "dve", pe_sem, gi + 1)
                dve.tensor_scalar(dst, ps[:, :tsz], scales[li], 0.0,
                                  mybir.AluOpType.mult,
                                  mybir.AluOpType.max).then_inc(dve_sem, 1)
            gidx[0] += 1

        # emission order, with gkeys tracking psum-slot owners
        order = []
        order += [(0, 0, o) for o in range(16)]
        order += [(1, 0, o) for o in range(16)]
        for o in range(8):      # A1 with C0 interleaved 2:1
            order += [(0, 1, o), (2, 0, 2 * o), (2, 0, 2 * o + 1)]
        for o in range(8):      # B1 with C1 interleaved 1:1
            order += [(1, 1, o), (2, 1, o)]
        # tail chunks: explicit engine split so neither engine queues two
        # consecutive full-width relus while the other idles
        for o, (ea, ec) in enumerate(zip("adad", "dada")):
            order += [(0, 2, o, ea), (2, 2, o, ec)]
        order += [(1, 2, o, e) for o, e in zip(range(4), "adad")]
        gkeys = [it[:3] for it in order]
        for it in order:
            group(*it[:3], eng=(it[3] if len(it) > 3 else None))

        NG = len(order)

        # ---- token-major L3 into ps3 columns ----
        def l3_i(ti, i):
            wait(pe, "pe", dsem[("w3", 0)], 16)
            wait(pe, "pe", zs_sem, 2)  # ps3 memset
            wait_relu(pe, "pe", (ti, 2, i))
            for (col, c_off, ncols) in tblocks[ti]:
                pe.matmul(ps3[0:ncols, col:col + 1],
                          h3[ti][:, i, c_off:c_off + ncols],
                          w3s[:, i, :], start=(i == 0), stop=(i == 3))

        for i in range(4):
            l3_i(0, i)
        for i in range(4):
            l3_i(2, i)
        for i in range(3):
            l3_i(1, i)
        wait(pe, "pe", dsem[("w3", 0)], 16)
        wait_relu(pe, "pe", (1, 2, 3))
        for (col, c_off, ncols) in tblocks[1]:
            last = col == tblocks[1][-1][0]
            mm = pe.matmul(ps3[0:ncols, col:col + 1],
                           h3[1][:, 3, c_off:c_off + ncols],
                           w3s[:, 3, :], start=False, stop=True)
            if last:
                mm.then_inc(pe_sem, 1)

        # ---- copy + trigger ----
        wait(act, "act", pe_sem, NG + 1)
        act.copy(outs[:, 0, 0, :], ps3[:, 0:NTB]).then_inc(copy_sem, 1)
        gp.wait_ge(prep_sem, 1)
        gp.wait_ge(copy_sem, 1)
        gp.trigger_dma(count=1)
        gp.wait_ge(odma_sem, 16)

    nc.finalize()
    return nc


def _build_graph(K, c0, c1, c2, ndummy=None, sched=None):
    """Build the SPMD Bass graph for capacity-K expert MLP on one core.

    c0..c2 are the descale factors folded into each layer's activation
    write (product of the input/weight pre-scales for that layer).
    """
    if ndummy is None:
        ndummy = NDUMMY
    if sched is None:
        sched = os.environ.get("KERNEL_SCHED", "V1")
    import concourse.bass as bass  # noqa: F401
    import concourse.tile as tile
    from concourse import bacc, mybir

    f8 = mybir.dt.float8e4
    bf = mybir.dt.bfloat16
    f32 = mybir.dt.float32
    AF = mybir.ActivationFunctionType
    DR = mybir.MatmulPerfMode.DoubleRow

    nc = bacc.Bacc("TRN2", target_bir_lowering=False, debug=False,
                   num_devices=NCORES)

    tiles = _token_tiles(K)
    nt = len(tiles)
    tblocks, NTB = _tile_tblocks(tiles)

    xT_d = [nc.declare_dram_parameter(f"xT{ti}", [P, 8, tsz], f8, False)
            for ti, (t0, tsz) in enumerate(tiles)]
    # weights in o-block-major layout: [n_blocks, 128, in_blocks, blk_ocols]
    wdt = [f8, f8, f8, bf]
    w_d = []
    for li, (ib, ob) in enumerate(_LAYER_BLOCKS):
        ocols = _WBLK_OCOLS[li]
        nblk = (ob * P) // ocols if li < 3 else 1
        w_d.append(nc.declare_dram_parameter(
            f"w{li}", [nblk, P, ib, ocols], wdt[li], False))
    out_d = nc.declare_dram_parameter("out", [P, NTB], f32, True)

    with tile.TileContext(nc) as tc:
        with (
            tc.tile_pool(name="wpool", bufs=1) as wpool,
            tc.tile_pool(name="xpool", bufs=1) as xpool,
            tc.tile_pool(name="hpool", bufs=1) as hpool,
            tc.tile_pool(name="opool", bufs=1) as opool,
            tc.tile_pool(name="psum", bufs=6, space="PSUM") as psum,
            tc.tile_pool(name="psumz", bufs=1, space="PSUM") as psumz,
            tc.tile_pool(name="psum3", bufs=1, space="PSUM") as psum3,
        ):
            # ---- warm-up: scratch memset + PE ramp ----
            zs = opool.tile([P, 2, 320], f8, tag="zs", name="zs")
            nc.vector.memset(zs[:], 0.0)
            zp = psumz.tile([P, 192], f32, tag="zp", name="zp")
            for _ in range(ndummy):
                nc.tensor.matmul(zp[:], zs[:, :, 0:128], zs[:, :, 128:320],
                                 start=True, stop=True, perf_mode=DR)

            outs = opool.tile([P, NTB], f32, tag="outs", name="outs")

            # ---- DMAs, in consumption order ----
            wblk = [[None] * ((ob * P) // _WBLK_OCOLS[li] if li < 3 else 1)
                    for li, (ib, ob) in enumerate(_LAYER_BLOCKS)]

            def load_wblock(li, blk):
                ib, ob = _LAYER_BLOCKS[li]
                ocols = _WBLK_OCOLS[li]
                t = wpool.tile([P, ib, ocols], wdt[li], tag=f"w{li}_{blk}",
                               name=f"w{li}_{blk}")
                nc.sync.dma_start(t[:], w_d[li][blk])
                wblk[li][blk] = t

            xs = [None] * nt

            def load_xtile(ti):
                t0, tsz = tiles[ti]
                t = xpool.tile([P, 8, tsz], f8, tag=f"xt_{ti}",
                               name=f"x_{ti}")
                nc.sync.dma_start(t[:], xT_d[ti][:])
                xs[ti] = t

            # longest transfer ordered after the first w0 block so the +900ns
            # DMA sem latencies overlap the following transfers
            load_wblock(0, 0)
            load_xtile(0)
            for blk in range(1, len(wblk[0])):
                load_wblock(0, blk)
            for ti in range(1, nt):
                load_xtile(ti)
            for blk in range(len(wblk[1])):
                load_wblock(1, blk)
            load_wblock(2, 0)
            load_wblock(3, 0)

            # ---- activations (h) and output tiles ----
            h1 = [hpool.tile([P, 16, tsz], f8, tag=f"h1_{ti}",
                             name=f"h1_{ti}")
                  for ti, (t0, tsz) in enumerate(tiles)]
            h2 = [hpool.tile([P, 8, tsz], f8, tag=f"h2_{ti}",
                             name=f"h2_{ti}")
                  for ti, (t0, tsz) in enumerate(tiles)]
            h3 = [hpool.tile([P, 4, tsz], bf, tag=f"h3_{ti}",
                             name=f"h3_{ti}")
                  for ti, (t0, tsz) in enumerate(tiles)]
            ps3 = psum3.tile([P, max(NTB, 2)], f32, tag="ps3", name="ps3")
            # column NTB-1 may have unwritten partitions (remainder tile):
            # keep PSUM defined for the full-width sigmoid read
            nc.vector.memset(ps3[:], 0.0)

            relu_cnt = [0]

            def relu(dst, src, scale, eng=None):
                if eng is None:
                    eng = "a" if relu_cnt[0] % 2 == 0 else "d"
                if eng == "a":
                    nc.scalar.activation(dst, src, AF.Relu, scale=scale)
                else:
                    nc.vector.tensor_scalar(dst, src, scale, 0.0,
                                            mybir.AluOpType.mult,
                                            mybir.AluOpType.max)
                relu_cnt[0] += 1

            scales = [c0, c1, c2]
            hts = [None, h1, h2, h3]

            def wslice(li, o, k2):
                """lhsT AP for out 128-block o, DoubleRow pair k2."""
                opb = _WBLK_OCOLS[li] // P
                t = wblk[li][o // opb]
                off = (o % opb) * P
                return t[:, 2 * k2:2 * k2 + 2, off:off + P]

            def group(ti, li, o, split=False, eng=None):
                """One 128-out-feature group: DR matmul chain + relu.

                split=True runs the relu as two half-token-range halves on
                both engines in parallel (lower latency for tail groups)."""
                t0, tsz = tiles[ti]
                npair = [4, 8, 4][li]
                ps = psum.tile([P, TT], f32, tag="ps",
                               name=f"ps{li}_{ti}_{o}")[:, :tsz]
                rhs_t = xs[ti] if li == 0 else hts[li][ti]
                for k in range(npair):
                    nc.tensor.matmul(ps, wslice(li, o, k),
                                     rhs_t[:, 2 * k:2 * k + 2, :tsz],
                                     start=(k == 0), stop=(k == npair - 1),
                                     perf_mode=DR)
                dst = hts[li + 1][ti][:, o, :tsz]
                if split and tsz >= 64:
                    h = (tsz // 2 + 15) // 16 * 16
                    s = scales[li]
                    nc.scalar.activation(dst[:, :h], ps[:, :h], AF.Relu,
                                         scale=s)
                    nc.vector.tensor_scalar(dst[:, h:], ps[:, h:], s, 0.0,
                                            mybir.AluOpType.mult,
                                            mybir.AluOpType.max)
                else:
                    relu(dst, ps, scales[li], eng=eng)

            def l3_i(ti, i):
                """Token-major final-layer matmuls for h3 i-block i of a
                tile: only waits the one producing relu; per-column
                accumulation still runs i=0..3 in order."""
                for (col, c_off, ncols) in tblocks[ti]:
                    nc.tensor.matmul(
                        ps3[0:ncols, col:col + 1],
                        h3[ti][:, i, c_off:c_off + ncols],
                        wblk[3][0][:, i, :],
                        start=(i == 0), stop=(i == 3))

            def l3_mms(ti):
                for i in range(4):
                    l3_i(ti, i)

            OB = [_LAYER_BLOCKS[li][1] for li in range(3)]  # 16, 8, 4

            if nt == 3 and tiles[2][1] <= 256:
                # wavefront schedule over tiles A, B and small remainder C:
                # each chunk's relu tail is covered by the next chunks'
                # matmul work before anything depends on it
                def chunk(ti, li, inter=None):
                    """Emit chunk (ti, li); inter interleaves small-tile
                    groups (list of (ti2, li2, o2)) 2 per big group."""
                    it = iter(inter or [])
                    for o in range(OB[li]):
                        group(ti, li, o)
                        for _ in range(2):
                            nxt = next(it, None)
                            if nxt is not None:
                                group(*nxt)
                    for nxt in it:
                        group(*nxt)

                c_groups = lambda li: [(2, li, o) for o in range(OB[li])]
                chunk(0, 0)
                chunk(1, 0)
                if sched == "V1":
                    chunk(0, 1, c_groups(0))
                    chunk(1, 1)
                    chunk(0, 2, c_groups(1))
                    chunk(1, 2)
                    chunk(2, 2)
                    l3_mms(0); l3_mms(1); l3_mms(2)
                elif sched == "V2":
                    chunk(0, 1, c_groups(0))
                    chunk(1, 1)
                    chunk(2, 1)
                    chunk(0, 2)
                    chunk(1, 2)
                    chunk(2, 2)
                    l3_mms(0); l3_mms(1); l3_mms(2)
                elif sched == "V3":
                    chunk(0, 1, c_groups(0))
                    chunk(1, 1, c_groups(1))
                    chunk(0, 2)
                    chunk(1, 2, c_groups(2))
                    l3_mms(0); l3_mms(1); l3_mms(2)
                elif sched == "V4":
                    chunk(0, 1, c_groups(0))
                    chunk(1, 1, c_groups(1))
                    chunk(0, 2)
                    l3_mms(0)
                    chunk(1, 2, c_groups(2))
                    l3_mms(1); l3_mms(2)
                elif sched == "V5":
                    chunk(0, 1, c_groups(0))
                    chunk(1, 1, c_groups(1))
                    chunk(2, 2)
                    chunk(0, 2)
                    l3_mms(2); l3_mms(0)
                    chunk(1, 2)
                    l3_mms(1)
                elif sched == "V6":
                    chunk(0, 1, c_groups(0))
                    chunk(1, 1, c_groups(1))
                    chunk(0, 2)
                    ci = iter(c_groups(2))
                    for o in range(OB[2]):
                        group(1, 2, o, split=True)
                        for _ in range(2):
                            nxt = next(ci, None)
                            if nxt is not None:
                                group(*nxt, split=True)
                    l3_mms(0)
                    for i in range(4):
                        l3_i(1, i)
                        l3_i(2, i)
                else:  # V8: C2 rides along A2 so the last chunk is B2
                    # alone; its relus alternate ending on the cheaper Act,
                    # and B's final layer is i-major to chase the relus
                    chunk(0, 1, c_groups(0))
                    chunk(1, 1, c_groups(1))
                    chunk(0, 2, c_groups(2))
                    for o, e in zip(range(OB[2]), "dada"):
                        group(1, 2, o, eng=e)
                    l3_mms(0)
                    l3_mms(2)
                    for i in range(4):
                        l3_i(1, i)
            else:
                # generic fallback: layer-major waves over all tiles
                for li in range(3):
                    for ti in range(nt):
                        for o in range(OB[li]):
                            group(ti, li, o)
                for ti in range(nt):
                    l3_mms(ti)

            # raw logits out (via SBUF); host applies the final sigmoid
            nc.scalar.copy(outs[:], ps3[:, 0:NTB])
            nc.sync.dma_start(out_d[:], outs[:])

    nc.finalize()
    return nc


def _np_dt(mdt_name):
    from concourse import mybir
    return mybir.dt.np(getattr(mybir.dt, mdt_name))


def _feature_major(a2d, npdt):
    """[T, F] -> SBUF layout [128, F//128, T] (contiguous)."""
    T, F = a2d.shape
    a = np.ascontiguousarray(a2d.T.reshape(F // P, P, T).transpose(1, 0, 2))
    return a.astype(npdt)


def _weight_blocked(wg, npdt, ocols):
    """[in, out] -> [n_blocks, 128, in_blocks, ocols] contiguous."""
    fin, fout = wg.shape
    ocols = min(ocols, fout)
    # blk[ob, p, i, oc] = wg[i*128+p, ob*ocols+oc]
    a = wg.reshape(fin // P, P, fout // ocols, ocols).transpose(2, 1, 0, 3)
    return np.ascontiguousarray(a).astype(npdt)


def kernel(x, center, w0_0, w0_1, w0_2, w0_3, wc_0, wc_1, wc_2, wc_3):
    from concourse.bass_utils import run_bass_kernel_spmd

    x = np.asarray(x, dtype=np.float32)
    center = np.asarray(center, dtype=np.float32)
    w0s = [np.asarray(w, dtype=np.float32) for w in (w0_0, w0_1, w0_2, w0_3)]
    wcs = [np.asarray(w, dtype=np.float32) for w in (wc_0, wc_1, wc_2, wc_3)]

    # --- host-side router + dispatch ---
    router = np.argmax(x @ center.T, axis=1)
    idxs = [np.where(router == c)[0] for c in range(C)]
    max_cnt = max(len(ix) for ix in idxs)
    K = max(P, int(math.ceil(max_cnt / 16)) * 16)

    # gated weights per cluster, and global per-layer fp8 pre-scales
    wg = [[w0s[li] * wcs[li][c] for c in range(C)] for li in range(4)]
    FP8_MAX = 240.0
    TINY = 1e-30
    ws = [max(TINY, max(np.abs(wg[li][c]).max() for c in range(C))) / FP8_MAX
          for li in range(3)]
    hs0 = max(TINY, np.abs(x).max()) / FP8_MAX

    # estimate activation ranges on a sample to pick gains G1, G2 that keep
    # stored fp8 activations well inside the normal range
    smp = x[:: max(1, B // 512)]
    m1 = m2 = 1e-9
    for c in range(C):
        a1 = np.maximum(smp @ wg[0][c], 0)
        m1 = max(m1, a1.max())
        a2 = np.maximum(a1 @ wg[1][c], 0)
        m2 = max(m2, a2.max())
    G1 = FP8_MAX / (8.0 * m1)
    G2 = FP8_MAX / (8.0 * m2)
    c0 = float(hs0 * ws[0] * G1)
    c1 = float(ws[1] * G2 / G1)
    c2 = float(ws[2] / G2)

    tiles_k = _token_tiles(K)
    use_raw = (os.environ.get("KERNEL_TILE", "0") != "1"
               and len(tiles_k) == 3 and tiles_k[2][1] <= 256)
    key = (use_raw, K, round(c0, 12), round(c1, 12), round(c2, 12), NDUMMY)
    if key not in _graph_cache:
        builder = _build_graph_raw if use_raw else _build_graph
        _graph_cache[key] = builder(K, c0, c1, c2)
    nc = _graph_cache[key]

    tiles = _token_tiles(K)
    tblocks, NTB = _tile_tblocks(tiles)

    f8np = _np_dt("float8e4")
    bfnp = _np_dt("bfloat16")
    in_maps = []
    for c in range(C):
        ix = idxs[c]
        xg = np.zeros((K, DIMS), np.float32)
        xg[:len(ix)] = x[ix] / hs0
        xf = _feature_major(xg, f8np)  # [128, 8, K]
        m = {}
        for ti, (t0, tsz) in enumerate(tiles):
            m[f"xT{ti}"] = np.ascontiguousarray(xf[:, :, t0:t0 + tsz])
        for li in range(3):
            m[f"w{li}"] = _weight_blocked(wg[li][c] / ws[li], f8np,
                                          _WBLK_OCOLS[li])
        m["w3"] = _weight_blocked(wg[3][c], bfnp, _WBLK_OCOLS[3])
        in_maps.append(m)

    import time

    res = None
    outs_np = None
    last_err = None
    for attempt in range(3):
        try:
            res = run_bass_kernel_spmd(nc, in_maps,
                                       core_ids=list(range(NCORES)))
            # force device->host readback here so transient faults retry
            outs_np = [np.asarray(res.results[c]["out"]) for c in range(C)]
            break
        except ModuleNotFoundError:
            # Axon stub without the NTFF profile hook: disable tracing.
            os.environ["BASS_NEVER_TRACE"] = "1"
        except Exception as e:  # transient device faults: retry
            last_err = e  # noqa: F841
            os.environ["NEURON_RT_RESET_CORES"] = "1"
            time.sleep(8.0 * (attempt + 1))
    if outs_np is None:
        res = run_bass_kernel_spmd(nc, in_maps, core_ids=list(range(NCORES)))
        outs_np = [np.asarray(res.results[c]["out"]) for c in range(C)]

    global last_run
    last_run = res

    out = np.zeros(B, np.float32)
    for c in range(C):
        ix = idxs[c]
        o = outs_np[c].reshape(P, NTB)  # raw logits
        vals = np.zeros(K, np.float32)
        for ti, (t0, tsz) in enumerate(tiles):
            for (col, c_off, ncols) in tblocks[ti]:
                vals[t0 + c_off:t0 + c_off + ncols] = o[:ncols, col]
        z = vals[:len(ix)]
        out[ix] = 1.0 / (1.0 + np.exp(-z))
    return out


# revision 52
# speedup vs baseline: 1.0030x; 1.0030x over previous
"""AdaptDHM MoE-routing kernel for one TRN2 chip (8 NeuronCores).

Strategy (expert-parallel dispatch, done host-side):
  - router = argmax(x @ center.T) picks one of C=8 clusters per token.
  - Each token is dispatched to the core owning its cluster; the core runs
    the 4-layer MLP (1024->2048->1024->512->1, relu; final sigmoid on
    host) once per token, padded to common capacity K (SPMD, one NEFF).
  - Compute: layers 0-2 in fp8-e4m3 with DoubleRow matmuls, layer 3 in
    bf16 token-major (tokens as lhsT columns, out free dim 1) so its PE
    cost is negligible and the output lands partition-major (tiny DMA).
  - Schedule (raw builder, manual semaphores, no Tile barriers): K split
    into (512, 512, rem) token tiles, layers software-pipelined in a
    wavefront (A0 B0 | A1+C0 | B1+C1 | A2+C2 | B2 | L3s) so the tensor
    engine never waits for a relu at a layer boundary. Warm-up matmuls on
    zeroed scratch keep the PE p-state ramp hot while the first DMAs
    land; weight/x DMAs are ordered to match consumption so the PE is
    supply-fed throughout. Relus alternate Scalar/Vector engines with an
    explicit balanced assignment in the tail chunks.
  - Output: SWDGE descriptors for the [128, NTB] logits are prepared at
    kernel start (kv_writeback prepare_only, ctx_idx=0 == plain
    SBUF->DRAM write); the end-of-kernel trigger skips the HWDGE+DGE
    issue latency on the critical tail.
  - Host scatters per-core logits back to the [B] output and applies the
    sigmoid.
"""

import math
import os

import ml_dtypes
import numpy as np

B, DIMS = 8192, 1024
FCN = [DIMS, 2048, 1024, 512, 1]
C = 8
NCORES = 8
P = 128
TT = 512  # max token tile (matmul moving free dim / PSUM bank)

_BF16 = ml_dtypes.bfloat16

_graph_cache = {}
last_run = None  # BassKernelResults of the most recent kernel() call

# per-layer (in_blocks, out_blocks)
_LAYER_BLOCKS = [(8, 16), (16, 8), (8, 4), (4, 1)]
# out columns per DMA-able weight block
_WBLK_OCOLS = [256, 128, 512, 1]

NDUMMY = int(os.environ.get("KERNEL_NDUMMY", "42"))


def _token_tiles(K):
    """Split K into full TT-sized tiles plus one remainder tile."""
    assert K % 16 == 0
    tiles = []
    t0 = 0
    while K - t0 >= TT:
        tiles.append((t0, TT))
        t0 += TT
    if K - t0 > 0:
        tiles.append((t0, K - t0))
    return tiles


def _tile_tblocks(tiles):
    """Token-blocks (<=128 tokens) per tile for the token-major L3.

    Returns list over tiles of list of (col, c_off, ncols); col is the
    global output column index in the [P, NTB] output."""
    out = []
    col = 0
    for (t0, tsz) in tiles:
        blocks = []
        c_off = 0
        while c_off < tsz:
            n = min(P, tsz - c_off)
            blocks.append((col, c_off, n))
            col += 1
            c_off += n
        out.append(blocks)
    return out, col


def _build_graph_raw(K, c0, c1, c2, ndummy=None):
    """Raw (no-Tile) builder: manual semaphores, no entry/exit barriers,
    SWDGE prepare/trigger output path. V8 wavefront schedule; requires
    nt == 3 with a small remainder tile."""
    import concourse.bass as bass  # noqa: F401
    from contextlib import ExitStack
    from concourse import bacc, mybir

    if ndummy is None:
        ndummy = NDUMMY
    f8 = mybir.dt.float8e4
    bf = mybir.dt.bfloat16
    f32 = mybir.dt.float32
    i32 = mybir.dt.int32
    AF = mybir.ActivationFunctionType
    DR = mybir.MatmulPerfMode.DoubleRow
    wdt = [f8, f8, f8, bf]

    nc = bacc.Bacc("TRN2", target_bir_lowering=False, debug=False,
                   num_devices=NCORES)

    tiles = _token_tiles(K)
    nt = len(tiles)
    assert nt == 3 and tiles[2][1] <= 256
    tblocks, NTB = _tile_tblocks(tiles)
    rem = tiles[2][1]

    xT_d = [nc.declare_dram_parameter(f"xT{ti}", [P, 8, tsz], f8, False)
            for ti, (t0, tsz) in enumerate(tiles)]
    w_d = []
    for li, (ib, ob) in enumerate(_LAYER_BLOCKS):
        ocols = _WBLK_OCOLS[li]
        nblk = (ob * P) // ocols if li < 3 else 1
        w_d.append(nc.declare_dram_parameter(
            f"w{li}", [nblk, P, ib, ocols], wdt[li], False))
    out_d = nc.declare_dram_parameter("out", [1, P, 1, NTB], f32, True)

    sp, pe, act, dve, gp = nc.sync, nc.tensor, nc.scalar, nc.vector, nc.gpsimd
    NPS = 6  # rotating psum banks

    with ExitStack() as ctx:
        sb = lambda name, shape, dt: ctx.enter_context(
            nc.sbuf_tensor(name, shape, dt))
        w0s = sb("w0s", [P, 8, 8, 256], f8)
        w1s = sb("w1s", [P, 8, 16, 128], f8)
        w2s = sb("w2s", [P, 8, 512], f8)
        w3s = sb("w3s", [P, 4, 1], bf)
        xsb = [sb(f"xs{ti}", [P, 8, tsz], f8)
               for ti, (t0, tsz) in enumerate(tiles)]
        h1 = [sb(f"h1_{ti}", [P, 16, tsz], f8)
              for ti, (t0, tsz) in enumerate(tiles)]
        h2 = [sb(f"h2_{ti}", [P, 8, tsz], f8)
              for ti, (t0, tsz) in enumerate(tiles)]
        h3 = [sb(f"h3_{ti}", [P, 4, tsz], bf)
              for ti, (t0, tsz) in enumerate(tiles)]
        outs = sb("outs", [P, 1, 1, NTB], f32)
        zs = sb("zs", [P, 2, 320], f8)
        kvidx = sb("kvidx", [P, 1], i32)
        banks = [ctx.enter_context(nc.psum_tensor(f"pb{i}", [P, TT], f32))
                 for i in range(NPS)]
        zp = ctx.enter_context(nc.psum_tensor("zp", [P, 192], f32))
        ps3 = ctx.enter_context(nc.psum_tensor("ps3", [P, 16], f32))

        sem = lambda name: ctx.enter_context(nc.semaphore(name))
        zs_sem = sem("zs")
        pe_sem = sem("pe")
        act_sem = sem("act")
        dve_sem = sem("dve")
        copy_sem = sem("copy")
        prep_sem = sem("prep")
        odma_sem = sem("odma")

        # ---- SP: input DMAs in consumption order ----
        dma_order = [("w0", 0), ("x", 0)]
        dma_order += [("w0", b) for b in range(1, 8)]
        dma_order += [("x", 1), ("x", 2)]
        dma_order += [("w1", b) for b in range(8)]
        dma_order += [("w2", 0), ("w3", 0)]
        dsem = {}
        for key in dma_order:
            kind, b = key
            dsem[key] = sem(f"dma_{kind}_{b}")
            if kind == "x":
                dst, src = xsb[b][:], xT_d[b][:]
            elif kind == "w0":
                dst, src = w0s[:, b], w_d[0][b]
            elif kind == "w1":
                dst, src = w1s[:, b], w_d[1][b]
            elif kind == "w2":
                dst, src = w2s[:], w_d[2][0]
            else:
                dst, src = w3s[:], w_d[3][0]
            sp.dma_start(out=dst, in_=src).then_inc(dsem[key], 16)

        # ---- Pool: output-path prep; trigger at the end ----
        gp.memset(kvidx[:], 0)
        gp.kv_writeback(out_d[:], outs[:], kvidx[:],
                        prepare_only=True, sem=odma_sem
                        ).then_inc(prep_sem, 1)

        # ---- DVE: scratch/ps3 memsets ----
        dve.memset(zs[:], 0.0).then_inc(zs_sem, 1)
        dve.memset(ps3[:], 0.0).then_inc(zs_sem, 1)

        # ---- PE warm-up ----
        pe.wait_ge(zs_sem, 1)
        for _ in range(ndummy):
            pe.matmul(zp[:], zs[:, :, 0:128], zs[:, :, 128:320],
                      start=True, stop=True, perf_mode=DR)

        # ---- main pipeline (V8 order) ----
        waited = {"pe": {}, "act": {}, "dve": {}}

        def wait(eng_obj, eng_key, s, v):
            w = waited[eng_key]
            if w.get(s.name, -1) < v:
                eng_obj.wait_ge(s, v)
                w[s.name] = v

        relu_cnt = [0]
        cnts = {"a": 0, "d": 0}
        group_relu = {}
        gidx = [0]
        hts = [None, h1, h2, h3]
        scales = [c0, c1, c2]

        def wslice(li, o, k):
            if li == 0:
                return w0s[:, o // 2, 2 * k:2 * k + 2,
                           (o % 2) * P:(o % 2) * P + P]
            if li == 1:
                return w1s[:, o, 2 * k:2 * k + 2, :]
            return w2s[:, 2 * k:2 * k + 2, o * P:(o + 1) * P]

        def wait_relu(eng_obj, eng_key, key):
            e, c = group_relu[key]
            if e == "s":
                wait(eng_obj, eng_key, act_sem, c[0])
                wait(eng_obj, eng_key, dve_sem, c[1])
            else:
                wait(eng_obj, eng_key, act_sem if e == "a" else dve_sem, c)

        def group(ti, li, o, eng=None):
            gi = gidx[0]
            t0_, tsz = tiles[ti]
            npair = [4, 8, 4][li]
            ps = banks[gi % NPS]
            # PSUM WAR: slot reused from group gi-NPS
            if gi >= NPS:
                wait_relu(pe, "pe", gkeys[gi - NPS])
            # weight/x DMA deps
            if li == 0:
                wait(pe, "pe", dsem[("x", ti)], 16)
                wait(pe, "pe", dsem[("w0", o // 2)], 16)
            elif li == 1:
                wait(pe, "pe", dsem[("w1", o)], 16)
            else:
                wait(pe, "pe", dsem[("w2", 0)], 16)
            rhs_t = xsb[ti] if li == 0 else hts[li][ti]
            for k in range(npair):
                if li > 0:
                    for bo in (2 * k, 2 * k + 1):
                        wait_relu(pe, "pe", (ti, li - 1, bo))
                mm = pe.matmul(ps[:, :tsz], wslice(li, o, k),
                               rhs_t[:, 2 * k:2 * k + 2, :tsz],
                               start=(k == 0), stop=(k == npair - 1),
                               perf_mode=DR)
                if k == npair - 1:
                    mm.then_inc(pe_sem, 1)
            # relu; eng "s" = token-split across both engines (low latency)
            if eng is None:
                eng = "a" if relu_cnt[0] % 2 == 0 else "d"
            relu_cnt[0] += 1
            dst = hts[li + 1][ti][:, o, :tsz]
            if eng == "s":
                h = tsz // 2 // 16 * 16
                cnts["a"] += 1
                cnts["d"] += 1
                group_relu[(ti, li, o)] = ("s", (cnts["a"], cnts["d"]))
                wait(act, "act", pe_sem, gi + 1)
                act.activation(dst[:, :h], ps[:, :h], AF.Relu,
                               scale=scales[li]).then_inc(act_sem, 1)
                wait(dve, "dve", pe_sem, gi + 1)
                dve.tensor_scalar(dst[:, h:tsz], ps[:, h:tsz], scales[li],
                                  0.0, mybir.AluOpType.mult,
                                  mybir.AluOpType.max).then_inc(dve_sem, 1)
            elif eng == "a":
                cnts[eng] += 1
                group_relu[(ti, li, o)] = (eng, cnts[eng])
                wait(act, "act", pe_sem, gi + 1)
                act.activation(dst, ps[:, :tsz], AF.Relu,
                               scale=scales[li]).then_inc(act_sem, 1)
            else:
                cnts[eng] += 1
                group_relu[(ti, li, o)] = (eng, cnts[eng])
                wait(dve, "dve", pe_sem, gi + 1)
                dve.tensor_scalar(dst, ps[:, :tsz], scales[li], 0.0,
                                  mybir.AluOpType.mult,
                                  mybir.AluOpType.max).then_inc(dve_sem, 1)
            gidx[0] += 1

        # emission order, with gkeys tracking psum-slot owners
        order = []
        order += [(0, 0, o) for o in range(16)]
        order += [(1, 0, o) for o in range(16)]
        for o in range(8):      # A1 with C0 interleaved 2:1
            order += [(0, 1, o), (2, 0, 2 * o), (2, 0, 2 * o + 1)]
        for o in range(8):      # B1 with C1 interleaved 1:1
            order += [(1, 1, o), (2, 1, o)]
        # tail chunks: explicit engine split so neither engine queues two
        # consecutive full-width relus while the other idles
        for o, (ea, ec) in enumerate(zip("adad", "dada")):
            order += [(0, 2, o, ea), (2, 2, o, ec)]
        order += [(1, 2, o, e) for o, e in zip(range(4), "adad")]
        gkeys = [it[:3] for it in order]
        for it in order:
            group(*it[:3], eng=(it[3] if len(it) > 3 else None))

        NG = len(order)

        # ---- token-major L3 into ps3 columns ----
        def l3_i(ti, i):
            wait(pe, "pe", dsem[("w3", 0)], 16)
            wait(pe, "pe", zs_sem, 2)  # ps3 memset
            wait_relu(pe, "pe", (ti, 2, i))
            for (col, c_off, ncols) in tblocks[ti]:
                pe.matmul(ps3[0:ncols, col:col + 1],
                          h3[ti][:, i, c_off:c_off + ncols],
                          w3s[:, i, :], start=(i == 0), stop=(i == 3))

        for i in range(4):
            l3_i(0, i)
        for i in range(4):
            l3_i(2, i)
        for i in range(3):
            l3_i(1, i)
        wait(pe, "pe", dsem[("w3", 0)], 16)
        wait_relu(pe, "pe", (1, 2, 3))
        for (col, c_off, ncols) in tblocks[1]:
            last = col == tblocks[1][-1][0]
            mm = pe.matmul(ps3[0:ncols, col:col + 1],
                           h3[1][:, 3, c_off:c_off + ncols],
                           w3s[:, 3, :], start=False, stop=True)
            if last:
                mm.then_inc(pe_sem, 1)

        # ---- copy + trigger (copy on DVE: lower access-latency const) ----
        wait(dve, "dve", pe_sem, NG + 1)
        dve.tensor_scalar(outs[:, 0, 0, :], ps3[:, 0:NTB], 1.0, 0.0,
                          mybir.AluOpType.mult,
                          mybir.AluOpType.add).then_inc(copy_sem, 1)
        gp.wait_ge(prep_sem, 1)
        gp.wait_ge(copy_sem, 1)
        gp.trigger_dma(count=1)
        gp.wait_ge(odma_sem, 16)

    nc.finalize()
    return nc


def _build_graph(K, c0, c1, c2, ndummy=None, sched=None):
    """Build the SPMD Bass graph for capacity-K expert MLP on one core.

    c0..c2 are the descale factors folded into each layer's activation
    write (product of the input/weight pre-scales for that layer).
    """
    if ndummy is None:
        ndummy = NDUMMY
    if sched is None:
        sched = os.environ.get("KERNEL_SCHED", "V1")
    import concourse.bass as bass  # noqa: F401
    import concourse.tile as tile
    from concourse import bacc, mybir

    f8 = mybir.dt.float8e4
    bf = mybir.dt.bfloat16
    f32 = mybir.dt.float32
    AF = mybir.ActivationFunctionType
    DR = mybir.MatmulPerfMode.DoubleRow

    nc = bacc.Bacc("TRN2", target_bir_lowering=False, debug=False,
                   num_devices=NCORES)

    tiles = _token_tiles(K)
    nt = len(tiles)
    tblocks, NTB = _tile_tblocks(tiles)

    xT_d = [nc.declare_dram_parameter(f"xT{ti}", [P, 8, tsz], f8, False)
            for ti, (t0, tsz) in enumerate(tiles)]
    # weights in o-block-major layout: [n_blocks, 128, in_blocks, blk_ocols]
    wdt = [f8, f8, f8, bf]
    w_d = []
    for li, (ib, ob) in enumerate(_LAYER_BLOCKS):
        ocols = _WBLK_OCOLS[li]
        nblk = (ob * P) // ocols if li < 3 else 1
        w_d.append(nc.declare_dram_parameter(
            f"w{li}", [nblk, P, ib, ocols], wdt[li], False))
    out_d = nc.declare_dram_parameter("out", [P, NTB], f32, True)

    with tile.TileContext(nc) as tc:
        with (
            tc.tile_pool(name="wpool", bufs=1) as wpool,
            tc.tile_pool(name="xpool", bufs=1) as xpool,
            tc.tile_pool(name="hpool", bufs=1) as hpool,
            tc.tile_pool(name="opool", bufs=1) as opool,
            tc.tile_pool(name="psum", bufs=6, space="PSUM") as psum,
            tc.tile_pool(name="psumz", bufs=1, space="PSUM") as psumz,
            tc.tile_pool(name="psum3", bufs=1, space="PSUM") as psum3,
        ):
            # ---- warm-up: scratch memset + PE ramp ----
            zs = opool.tile([P, 2, 320], f8, tag="zs", name="zs")
            nc.vector.memset(zs[:], 0.0)
            zp = psumz.tile([P, 192], f32, tag="zp", name="zp")
            for _ in range(ndummy):
                nc.tensor.matmul(zp[:], zs[:, :, 0:128], zs[:, :, 128:320],
                                 start=True, stop=True, perf_mode=DR)

            outs = opool.tile([P, NTB], f32, tag="outs", name="outs")

            # ---- DMAs, in consumption order ----
            wblk = [[None] * ((ob * P) // _WBLK_OCOLS[li] if li < 3 else 1)
                    for li, (ib, ob) in enumerate(_LAYER_BLOCKS)]

            def load_wblock(li, blk):
                ib, ob = _LAYER_BLOCKS[li]
                ocols = _WBLK_OCOLS[li]
                t = wpool.tile([P, ib, ocols], wdt[li], tag=f"w{li}_{blk}",
                               name=f"w{li}_{blk}")
                nc.sync.dma_start(t[:], w_d[li][blk])
                wblk[li][blk] = t

            xs = [None] * nt

            def load_xtile(ti):
                t0, tsz = tiles[ti]
                t = xpool.tile([P, 8, tsz], f8, tag=f"xt_{ti}",
                               name=f"x_{ti}")
                nc.sync.dma_start(t[:], xT_d[ti][:])
                xs[ti] = t

            # longest transfer ordered after the first w0 block so the +900ns
            # DMA sem latencies overlap the following transfers
            load_wblock(0, 0)
            load_xtile(0)
            for blk in range(1, len(wblk[0])):
                load_wblock(0, blk)
            for ti in range(1, nt):
                load_xtile(ti)
            for blk in range(len(wblk[1])):
                load_wblock(1, blk)
            load_wblock(2, 0)
            load_wblock(3, 0)

            # ---- activations (h) and output tiles ----
            h1 = [hpool.tile([P, 16, tsz], f8, tag=f"h1_{ti}",
                             name=f"h1_{ti}")
                  for ti, (t0, tsz) in enumerate(tiles)]
            h2 = [hpool.tile([P, 8, tsz], f8, tag=f"h2_{ti}",
                             name=f"h2_{ti}")
                  for ti, (t0, tsz) in enumerate(tiles)]
            h3 = [hpool.tile([P, 4, tsz], bf, tag=f"h3_{ti}",
                             name=f"h3_{ti}")
                  for ti, (t0, tsz) in enumerate(tiles)]
            ps3 = psum3.tile([P, max(NTB, 2)], f32, tag="ps3", name="ps3")
            # column NTB-1 may have unwritten partitions (remainder tile):
            # keep PSUM defined for the full-width sigmoid read
            nc.vector.memset(ps3[:], 0.0)

            relu_cnt = [0]

            def relu(dst, src, scale, eng=None):
                if eng is None:
                    eng = "a" if relu_cnt[0] % 2 == 0 else "d"
                if eng == "a":
                    nc.scalar.activation(dst, src, AF.Relu, scale=scale)
                else:
                    nc.vector.tensor_scalar(dst, src, scale, 0.0,
                                            mybir.AluOpType.mult,
                                            mybir.AluOpType.max)
                relu_cnt[0] += 1

            scales = [c0, c1, c2]
            hts = [None, h1, h2, h3]

            def wslice(li, o, k2):
                """lhsT AP for out 128-block o, DoubleRow pair k2."""
                opb = _WBLK_OCOLS[li] // P
                t = wblk[li][o // opb]
                off = (o % opb) * P
                return t[:, 2 * k2:2 * k2 + 2, off:off + P]

            def group(ti, li, o, split=False, eng=None):
                """One 128-out-feature group: DR matmul chain + relu.

                split=True runs the relu as two half-token-range halves on
                both engines in parallel (lower latency for tail groups)."""
                t0, tsz = tiles[ti]
                npair = [4, 8, 4][li]
                ps = psum.tile([P, TT], f32, tag="ps",
                               name=f"ps{li}_{ti}_{o}")[:, :tsz]
                rhs_t = xs[ti] if li == 0 else hts[li][ti]
                for k in range(npair):
                    nc.tensor.matmul(ps, wslice(li, o, k),
                                     rhs_t[:, 2 * k:2 * k + 2, :tsz],
                                     start=(k == 0), stop=(k == npair - 1),
                                     perf_mode=DR)
                dst = hts[li + 1][ti][:, o, :tsz]
                if split and tsz >= 64:
                    h = (tsz // 2 + 15) // 16 * 16
                    s = scales[li]
                    nc.scalar.activation(dst[:, :h], ps[:, :h], AF.Relu,
                                         scale=s)
                    nc.vector.tensor_scalar(dst[:, h:], ps[:, h:], s, 0.0,
                                            mybir.AluOpType.mult,
                                            mybir.AluOpType.max)
                else:
                    relu(dst, ps, scales[li], eng=eng)

            def l3_i(ti, i):
                """Token-major final-layer matmuls for h3 i-block i of a
                tile: only waits the one producing relu; per-column
                accumulation still runs i=0..3 in order."""
                for (col, c_off, ncols) in tblocks[ti]:
                    nc.tensor.matmul(
                        ps3[0:ncols, col:col + 1],
                        h3[ti][:, i, c_off:c_off + ncols],
                        wblk[3][0][:, i, :],
                        start=(i == 0), stop=(i == 3))

            def l3_mms(ti):
                for i in range(4):
                    l3_i(ti, i)

            OB = [_LAYER_BLOCKS[li][1] for li in range(3)]  # 16, 8, 4

            if nt == 3 and tiles[2][1] <= 256:
                # wavefront schedule over tiles A, B and small remainder C:
                # each chunk's relu tail is covered by the next chunks'
                # matmul work before anything depends on it
                def chunk(ti, li, inter=None):
                    """Emit chunk (ti, li); inter interleaves small-tile
                    groups (list of (ti2, li2, o2)) 2 per big group."""
                    it = iter(inter or [])
                    for o in range(OB[li]):
                        group(ti, li, o)
                        for _ in range(2):
                            nxt = next(it, None)
                            if nxt is not None:
                                group(*nxt)
                    for nxt in it:
                        group(*nxt)

                c_groups = lambda li: [(2, li, o) for o in range(OB[li])]
                chunk(0, 0)
                chunk(1, 0)
                if sched == "V1":
                    chunk(0, 1, c_groups(0))
                    chunk(1, 1)
                    chunk(0, 2, c_groups(1))
                    chunk(1, 2)
                    chunk(2, 2)
                    l3_mms(0); l3_mms(1); l3_mms(2)
                elif sched == "V2":
                    chunk(0, 1, c_groups(0))
                    chunk(1, 1)
                    chunk(2, 1)
                    chunk(0, 2)
                    chunk(1, 2)
                    chunk(2, 2)
                    l3_mms(0); l3_mms(1); l3_mms(2)
                elif sched == "V3":
                    chunk(0, 1, c_groups(0))
                    chunk(1, 1, c_groups(1))
                    chunk(0, 2)
                    chunk(1, 2, c_groups(2))
                    l3_mms(0); l3_mms(1); l3_mms(2)
                elif sched == "V4":
                    chunk(0, 1, c_groups(0))
                    chunk(1, 1, c_groups(1))
                    chunk(0, 2)
                    l3_mms(0)
                    chunk(1, 2, c_groups(2))
                    l3_mms(1); l3_mms(2)
                elif sched == "V5":
                    chunk(0, 1, c_groups(0))
                    chunk(1, 1, c_groups(1))
                    chunk(2, 2)
                    chunk(0, 2)
                    l3_mms(2); l3_mms(0)
                    chunk(1, 2)
                    l3_mms(1)
                elif sched == "V6":
                    chunk(0, 1, c_groups(0))
                    chunk(1, 1, c_groups(1))
                    chunk(0, 2)
                    ci = iter(c_groups(2))
                    for o in range(OB[2]):
                        group(1, 2, o, split=True)
                        for _ in range(2):
                            nxt = next(ci, None)
                            if nxt is not None:
                                group(*nxt, split=True)
                    l3_mms(0)
                    for i in range(4):
                        l3_i(1, i)
                        l3_i(2, i)
                else:  # V8: C2 rides along A2 so the last chunk is B2
                    # alone; its relus alternate ending on the cheaper Act,
                    # and B's final layer is i-major to chase the relus
                    chunk(0, 1, c_groups(0))
                    chunk(1, 1, c_groups(1))
                    chunk(0, 2, c_groups(2))
                    for o, e in zip(range(OB[2]), "dada"):
                        group(1, 2, o, eng=e)
                    l3_mms(0)
                    l3_mms(2)
                    for i in range(4):
                        l3_i(1, i)
            else:
                # generic fallback: layer-major waves over all tiles
                for li in range(3):
                    for ti in range(nt):
                        for o in range(OB[li]):
                            group(ti, li, o)
                for ti in range(nt):
                    l3_mms(ti)

            # raw logits out (via SBUF); host applies the final sigmoid
            nc.scalar.copy(outs[:], ps3[:, 0:NTB])
            nc.sync.dma_start(out_d[:], outs[:])

    nc.finalize()
    return nc


def _np_dt(mdt_name):
    from concourse import mybir
    return mybir.dt.np(getattr(mybir.dt, mdt_name))


def _feature_major(a2d, npdt):
    """[T, F] -> SBUF layout [128, F//128, T] (contiguous)."""
    T, F = a2d.shape
    a = np.ascontiguousarray(a2d.T.reshape(F // P, P, T).transpose(1, 0, 2))
    return a.astype(npdt)


def _weight_blocked(wg, npdt, ocols):
    """[in, out] -> [n_blocks, 128, in_blocks, ocols] contiguous."""
    fin, fout = wg.shape
    ocols = min(ocols, fout)
    # blk[ob, p, i, oc] = wg[i*128+p, ob*ocols+oc]
    a = wg.reshape(fin // P, P, fout // ocols, ocols).transpose(2, 1, 0, 3)
    return np.ascontiguousarray(a).astype(npdt)


def kernel(x, center, w0_0, w0_1, w0_2, w0_3, wc_0, wc_1, wc_2, wc_3):
    from concourse.bass_utils import run_bass_kernel_spmd

    x = np.asarray(x, dtype=np.float32)
    center = np.asarray(center, dtype=np.float32)
    w0s = [np.asarray(w, dtype=np.float32) for w in (w0_0, w0_1, w0_2, w0_3)]
    wcs = [np.asarray(w, dtype=np.float32) for w in (wc_0, wc_1, wc_2, wc_3)]

    # --- host-side router + dispatch ---
    router = np.argmax(x @ center.T, axis=1)
    idxs = [np.where(router == c)[0] for c in range(C)]
    max_cnt = max(len(ix) for ix in idxs)
    K = max(P, int(math.ceil(max_cnt / 16)) * 16)

    # gated weights per cluster, and global per-layer fp8 pre-scales
    wg = [[w0s[li] * wcs[li][c] for c in range(C)] for li in range(4)]
    FP8_MAX = 240.0
    TINY = 1e-30
    ws = [max(TINY, max(np.abs(wg[li][c]).max() for c in range(C))) / FP8_MAX
          for li in range(3)]
    hs0 = max(TINY, np.abs(x).max()) / FP8_MAX

    # estimate activation ranges on a sample to pick gains G1, G2 that keep
    # stored fp8 activations well inside the normal range
    smp = x[:: max(1, B // 512)]
    m1 = m2 = 1e-9
    for c in range(C):
        a1 = np.maximum(smp @ wg[0][c], 0)
        m1 = max(m1, a1.max())
        a2 = np.maximum(a1 @ wg[1][c], 0)
        m2 = max(m2, a2.max())
    G1 = FP8_MAX / (8.0 * m1)
    G2 = FP8_MAX / (8.0 * m2)
    c0 = float(hs0 * ws[0] * G1)
    c1 = float(ws[1] * G2 / G1)
    c2 = float(ws[2] / G2)

    tiles_k = _token_tiles(K)
    use_raw = (os.environ.get("KERNEL_TILE", "0") != "1"
               and len(tiles_k) == 3 and tiles_k[2][1] <= 256)
    key = (use_raw, K, round(c0, 12), round(c1, 12), round(c2, 12), NDUMMY)
    if key not in _graph_cache:
        builder = _build_graph_raw if use_raw else _build_graph
        _graph_cache[key] = builder(K, c0, c1, c2)
    nc = _graph_cache[key]

    tiles = _token_tiles(K)
    tblocks, NTB = _tile_tblocks(tiles)

    f8np = _np_dt("float8e4")
    bfnp = _np_dt("bfloat16")
    in_maps = []
    for c in range(C):
        ix = idxs[c]
        xg = np.zeros((K, DIMS), np.float32)
        xg[:len(ix)] = x[ix] / hs0
        xf = _feature_major(xg, f8np)  # [128, 8, K]
        m = {}
        for ti, (t0, tsz) in enumerate(tiles):
            m[f"xT{ti}"] = np.ascontiguousarray(xf[:, :, t0:t0 + tsz])
        for li in range(3):
            m[f"w{li}"] = _weight_blocked(wg[li][c] / ws[li], f8np,
                                          _WBLK_OCOLS[li])
        m["w3"] = _weight_blocked(wg[3][c], bfnp, _WBLK_OCOLS[3])
        in_maps.append(m)

    import time

    res = None
    outs_np = None
    last_err = None
    for attempt in range(3):
        try:
            res = run_bass_kernel_spmd(nc, in_maps,
                                       core_ids=list(range(NCORES)))
            # force device->host readback here so transient faults retry
            outs_np = [np.asarray(res.results[c]["out"]) for c in range(C)]
            break
        except ModuleNotFoundError:
            # Axon stub without the NTFF profile hook: disable tracing.
            os.environ["BASS_NEVER_TRACE"] = "1"
        except Exception as e:  # transient device faults: retry
            last_err = e  # noqa: F841
            os.environ["NEURON_RT_RESET_CORES"] = "1"
            time.sleep(8.0 * (attempt + 1))
    if outs_np is None:
        res = run_bass_kernel_spmd(nc, in_maps, core_ids=list(range(NCORES)))
        outs_np = [np.asarray(res.results[c]["out"]) for c in range(C)]

    global last_run
    last_run = res

    out = np.zeros(B, np.float32)
    for c in range(C):
        ix = idxs[c]
        o = outs_np[c].reshape(P, NTB)  # raw logits
        vals = np.zeros(K, np.float32)
        for ti, (t0, tsz) in enumerate(tiles):
            for (col, c_off, ncols) in tblocks[ti]:
                vals[t0 + c_off:t0 + c_off + ncols] = o[:ncols, col]
        z = vals[:len(ix)]
        out[ix] = 1.0 / (1.0 + np.exp(-z))
    return out
